# revision 2
# baseline (speedup 1.0000x reference)
"""5G Polar encoder on 8 trn2 cores: one fused GF(2) matmul.

The whole reference computation is GF(2)-linear in u, so the host
composes one binary matrix M [512, 1024] from the tiny index tables and
the device computes y = (u @ M) mod 2, data-parallel over the batch
(8192 rows/core), as an fp8e4 DoubleRow matmul accumulating in f32 PSUM
(exact: sums <= 512), with mod-2 on the eviction path (ACT f32->i16,
DVE AND 1) and i16 {0,1} DMA'd out (host converts to f32).

Per-pass engine budget (controlled reps-difference measurements):
  PE:  256 (LDWEIGHTS+MATMUL) pairs ~74us -- the wall.  512-col moving
       streams at 1 col/cycle (DoubleRow halves contraction passes, not
       column rate) and DR disables FWL so each LDW exposes ~80ns.
       Matmul order alternates PSUM banks (same-bank RMW back-to-back
       stalls ~120ns/mm) and keeps the stationary operand for 2 mms.
  ACT+DVE eviction ~45us, output DMA 16MB ~45us: hidden under PE.
  Input u loads in 4 chunks (2KB DMA segments; 1KB segments at 8 chunks
  measured ~2x slower) and overlaps the previous tiles' matmuls.
"""

import numpy as np
import ml_dtypes

N_CORES = 8
BS = 65536
K = 512          # u feature dim (contraction)
N = 1024         # output columns
SHARD = BS // N_CORES  # 8192 batch rows per core
P = 128
KT = K // P      # 4 k-tiles
NB = SHARD // P  # 64 batch tiles per core

FP8_NP = ml_dtypes.float8_e4m3

_nc_cache = {}


def build_M(crc_gen, info_pos, ind_gather, perm_out):
    """Compose the encoder into one GF(2) matrix M [K, N]: out = (u @ M) mod 2."""
    crc_gen = np.asarray(crc_gen)
    info_pos = np.asarray(info_pos)
    ind_gather = np.asarray(ind_gather)
    perm_out = np.asarray(perm_out)
    k, _ = crc_gen.shape
    nb, n1 = ind_gather.shape
    kp = info_pos.shape[0]
    C = (crc_gen.astype(np.int64) & 1).astype(np.uint8)
    B = np.concatenate([np.eye(k, dtype=np.uint8), C], axis=1)  # [k, kp]
    # scatter bits into columns; duplicate indices: last write wins
    col_src = np.full(n1, -1, np.int64)
    col_src[info_pos] = np.arange(kp)
    A = np.zeros((k, n1), np.uint8)
    valid = col_src >= 0
    A[:, valid] = B[:, col_src[valid]]
    for s in range(nb):
        A = A ^ A[:, ind_gather[s]]
    return A[:, perm_out]  # [k, n]


def _build_nc(reps=1, w1_act=48, u_chunks=4, wbufs=6, uload_in_reps=False):
    """reps>1 repeats the whole per-execute body (for slope benchmarks);
    uload_in_reps puts the u/mat DMA inside the rep loop so the marginal
    rep includes the full input load."""
    import concourse.tile as tile
    from concourse import bacc, mybir

    nc = bacc.Bacc("TRN2", target_bir_lowering=False, debug=False)
    fp8 = mybir.dt.float8e4
    f32 = mybir.dt.float32
    i16 = mybir.dt.int16
    DR = mybir.MatmulPerfMode.DoubleRow

    # k-major 3D layouts: [p, ks, free] with global k = ks*128 + p (both
    # operands use the same mapping, so the contraction is correct).
    uT = nc.declare_dram_parameter("uT", [P, KT, SHARD], fp8, isOutput=False)
    mat = nc.declare_dram_parameter("mat", [P, KT, N], fp8, isOutput=False)
    y = nc.declare_dram_parameter("y", [SHARD, N], i16, isOutput=True)

    CW = SHARD // u_chunks
    with tile.TileContext(nc) as tc:
        with (
            tc.tile_pool(name="consts", bufs=1) as cpool,
            tc.tile_pool(name="work", bufs=wbufs) as wpool,
            tc.tile_pool(name="outs", bufs=4) as opool,
            tc.tile_pool(name="psum", bufs=4, space="PSUM") as ppool,
        ):
            mt = cpool.tile([P, KT, N], fp8, tag="mt")
            uts = [
                cpool.tile([P, KT, CW], fp8, tag=f"ut{c}", name=f"ut{c}")
                for c in range(u_chunks)
            ]
            for r in range(reps):
                if r == 0 or uload_in_reps:
                    nc.sync.dma_start(mt[:], mat[:])
                    for c in range(u_chunks):
                        nc.sync.dma_start(
                            uts[c][:], uT[:, :, c * CW:(c + 1) * CW]
                        )
                for b in range(NB):
                    i = r * NB + b
                    ps = ppool.tile([P, N], f32, tag="ps", name="ps")
                    ut = uts[(b * P) // CW]
                    boff = (b * P) % CW
                    # ks-outer: one stationary (LDWEIGHTS) serves both
                    # psum halves; halves alternate PSUM banks so no
                    # back-to-back same-bank RMW.
                    for ks in (0, 2):
                        for h in range(2):
                            nc.tensor.matmul(
                                ps[:, h * 512:(h + 1) * 512],
                                ut[:, ks:ks + 2, boff:boff + P],
                                mt[:, ks:ks + 2, h * 512:(h + 1) * 512],
                                start=(ks == 0),
                                stop=(ks == 2),
                                perf_mode=DR,
                                skip_group_check=True,
                            )
                    t16 = wpool.tile([P, N], i16, tag="t16")
                    a16 = wpool.tile([P, N], i16, tag="a16")
                    # W1: PSUM f32 -> i16 (values <= 512 fit exactly);
                    # split ACT/DVE to balance engine time
                    if (i % NB) < w1_act:
                        nc.scalar.activation(
                            t16[:], ps[:],
                            mybir.ActivationFunctionType.Copy,
                        )
                    else:
                        nc.vector.tensor_copy(t16[:], ps[:])
                    # W2: AND with 1 (i16 4x mode)
                    nc.vector.tensor_scalar(
                        a16[:], t16[:], 1, None,
                        mybir.AluOpType.bitwise_and,
                    )
                    nc.sync.dma_start(y[b * P:(b + 1) * P, :], a16[:])
    nc.compile()
    return nc


W1_ACT = 48
U_CHUNKS = 4
WBUFS = 6


def get_nc(reps=1, uload_in_reps=False):
    key = (reps, W1_ACT, U_CHUNKS, WBUFS, uload_in_reps)
    if key not in _nc_cache:
        _nc_cache[key] = _build_nc(reps, w1_act=W1_ACT, u_chunks=U_CHUNKS,
                                   wbufs=WBUFS, uload_in_reps=uload_in_reps)
    return _nc_cache[key]


def _to_k_major(a_km, free):
    """[K, free] -> [P, KT, free] with k = ks*128 + p."""
    return np.ascontiguousarray(
        a_km.reshape(KT, P, free).transpose(1, 0, 2)
    )


def make_in_maps(u, M):
    u8 = np.asarray(u).astype(FP8_NP)
    m8 = np.asarray(M).astype(FP8_NP)
    mat3 = _to_k_major(m8, N)
    in_maps = []
    for i in range(N_CORES):
        uT_i = np.ascontiguousarray(u8[i * SHARD:(i + 1) * SHARD, :].T)
        in_maps.append({"uT": _to_k_major(uT_i, SHARD), "mat": mat3})
    return in_maps


def kernel(u, crc_gen, info_pos, ind_gather, perm_out):
    from concourse.bass_utils import run_bass_kernel_spmd

    M = build_M(crc_gen, info_pos, ind_gather, perm_out)
    in_maps = make_in_maps(u, M)
    nc = get_nc()
    res = run_bass_kernel_spmd(nc, in_maps, core_ids=list(range(N_CORES)))
    out = np.concatenate(
        [np.asarray(r["y"]).astype(np.float32) for r in res.results], axis=0
    )
    return out


# revision 3
# speedup vs baseline: 1.0587x; 1.0587x over previous
"""5G Polar encoder on 8 trn2 cores: one fused GF(2) matmul,
DoubleRowSwInterleave weights.

The whole reference computation is GF(2)-linear in u, so the host
composes one binary matrix M [512, 1024] from the tiny index tables and
the device computes y = (u @ M) mod 2, data-parallel over the batch
(8192 rows/core), as an fp8e4 matmul accumulating in f32 PSUM (exact:
sums <= 512), with mod-2 on the eviction path (ACT/DVE f32->i16, DVE
AND 1) and i16 {0,1} DMA'd out (host converts to f32).

Matmul perf mode is DoubleRowSwInterleave: the stationary operand is
stored flat [p, 256] with the two DR k-row-sets interleaved per column
and columns REVERSED (bass_interp.py:5260):
    F[p, 2*(127-m)]   = W0[p, m]   (k-row-set A = ks)
    F[p, 2*(127-m)+1] = W1[p, m]   (k-row-set B = ks+1)
so the PE reads weights contiguously.  Measured: the matmul stage drops
from ~75us (plain DoubleRow, LDWEIGHTS exposes ~80ns/mm since DR
disables fast weight load) to ~60-62us; full kernel ~76us vs ~79us.
Matmul order alternates PSUM banks (same-bank RMW back-to-back stalls)
and the stationary serves both psum halves.  Eviction split ACT 44 /
DVE 20 tiles balances both engines at ~45us, hidden under PE.  i8
output (extra DVE narrow) measured WORSE (80us) - engine time, not
HBM, is the secondary constraint.
"""

import numpy as np
import ml_dtypes

N_CORES = 8
BS = 65536
K = 512
N = 1024
SHARD = BS // N_CORES
P = 128
KT = K // P
NB = SHARD // P

FP8_NP = ml_dtypes.float8_e4m3

_nc_cache = {}


def build_M(crc_gen, info_pos, ind_gather, perm_out):
    crc_gen = np.asarray(crc_gen)
    info_pos = np.asarray(info_pos)
    ind_gather = np.asarray(ind_gather)
    perm_out = np.asarray(perm_out)
    k, _ = crc_gen.shape
    nb, n1 = ind_gather.shape
    kp = info_pos.shape[0]
    C = (crc_gen.astype(np.int64) & 1).astype(np.uint8)
    B = np.concatenate([np.eye(k, dtype=np.uint8), C], axis=1)
    col_src = np.full(n1, -1, np.int64)
    col_src[info_pos] = np.arange(kp)
    A = np.zeros((k, n1), np.uint8)
    valid = col_src >= 0
    A[:, valid] = B[:, col_src[valid]]
    for s in range(nb):
        A = A ^ A[:, ind_gather[s]]
    return A[:, perm_out]


def _build_nc(reps=1, w1_act=48, u_chunks=4, wbufs=6, uload_in_reps=False,
              swi=True, rev=True, variant="full", evict8=False):
    import concourse.tile as tile
    from concourse import bacc, mybir

    nc = bacc.Bacc("TRN2", target_bir_lowering=False, debug=False)
    fp8 = mybir.dt.float8e4
    f32 = mybir.dt.float32
    i16 = mybir.dt.int16
    i8 = mybir.dt.int8
    MODE = (mybir.MatmulPerfMode.DoubleRowSwInterleave if swi
            else mybir.MatmulPerfMode.DoubleRow)

    if swi:
        uT = nc.declare_dram_parameter("uT", [P, 2, NB, 2 * P], fp8,
                                       isOutput=False)
    else:
        uT = nc.declare_dram_parameter("uT", [P, KT, SHARD], fp8,
                                       isOutput=False)
    mat = nc.declare_dram_parameter("mat", [P, KT, N], fp8, isOutput=False)
    y = nc.declare_dram_parameter("y", [SHARD, N], i8 if evict8 else i16,
                                  isOutput=True)

    NBC = NB // u_chunks   # batch tiles per chunk (swi layout)
    CW = SHARD // u_chunks
    with tile.TileContext(nc) as tc:
        with (
            tc.tile_pool(name="consts", bufs=1) as cpool,
            tc.tile_pool(name="work", bufs=wbufs) as wpool,
            tc.tile_pool(name="outs", bufs=6) as opool,
            tc.tile_pool(name="psum", bufs=4, space="PSUM") as ppool,
        ):
            mt = cpool.tile([P, KT, N], fp8, tag="mt")
            if swi:
                uts = [
                    cpool.tile([P, 2, NBC, 2 * P], fp8, tag=f"ut{c}",
                               name=f"ut{c}")
                    for c in range(u_chunks)
                ]
            else:
                uts = [
                    cpool.tile([P, KT, CW], fp8, tag=f"ut{c}", name=f"ut{c}")
                    for c in range(u_chunks)
                ]
            for r in range(reps):
                if r == 0 or uload_in_reps:
                    nc.sync.dma_start(mt[:], mat[:])
                    for c in range(u_chunks):
                        if swi:
                            nc.sync.dma_start(
                                uts[c][:],
                                uT[:, :, c * NBC:(c + 1) * NBC, :])
                        else:
                            nc.sync.dma_start(
                                uts[c][:], uT[:, :, c * CW:(c + 1) * CW])
                for b in range(NB):
                    i = r * NB + b
                    ps = ppool.tile([P, N], f32, tag="ps", name="ps")
                    if swi:
                        ut = uts[b // NBC]
                        bl = b % NBC
                    else:
                        ut = uts[(b * P) // CW]
                        boff = (b * P) % CW
                    for ks in (0, 2):
                        g = ks // 2
                        for h in range(2):
                            lhsT = (ut[:, g, bl, :] if swi
                                    else ut[:, ks:ks + 2, boff:boff + P])
                            nc.tensor.matmul(
                                ps[:, h * 512:(h + 1) * 512],
                                lhsT,
                                mt[:, ks:ks + 2, h * 512:(h + 1) * 512],
                                start=(ks == 0),
                                stop=(ks == 2),
                                perf_mode=MODE,
                                skip_group_check=True,
                            )
                    if variant == "mm":
                        continue
                    t16 = wpool.tile([P, N], i16, tag="t16")
                    a16 = wpool.tile([P, N], i16, tag="a16")
                    if (i % NB) < w1_act:
                        nc.scalar.activation(
                            t16[:], ps[:],
                            mybir.ActivationFunctionType.Copy,
                        )
                    else:
                        nc.vector.tensor_copy(t16[:], ps[:])
                    nc.vector.tensor_scalar(
                        a16[:], t16[:], 1, None,
                        mybir.AluOpType.bitwise_and,
                    )
                    if evict8:
                        o8 = opool.tile([P, N], i8, tag="o8")
                        nc.vector.tensor_copy(o8[:], a16[:])
                        nc.sync.dma_start(y[b * P:(b + 1) * P, :], o8[:])
                    else:
                        nc.sync.dma_start(y[b * P:(b + 1) * P, :], a16[:])
    nc.compile()
    return nc


W1_ACT = 44
U_CHUNKS = 4
WBUFS = 8
SWI = True
REV = True
EVICT8 = False


def get_nc(reps=1, uload_in_reps=False, variant="full"):
    key = (reps, W1_ACT, U_CHUNKS, WBUFS, uload_in_reps, SWI, REV, variant,
           EVICT8)
    if key not in _nc_cache:
        _nc_cache[key] = _build_nc(reps, w1_act=W1_ACT, u_chunks=U_CHUNKS,
                                   wbufs=WBUFS, uload_in_reps=uload_in_reps,
                                   swi=SWI, rev=REV, variant=variant,
                                   evict8=EVICT8)
    return _nc_cache[key]


def _to_k_major(a_km, free):
    return np.ascontiguousarray(
        a_km.reshape(KT, P, free).transpose(1, 0, 2)
    )


def make_swi(u3, rev=True):
    """[P, KT, SHARD] -> [P, 2, NB, 256] SwInterleave stationary layout."""
    u4 = u3.reshape(P, KT, NB, P)
    swi = np.empty((P, 2, NB, 2 * P), u3.dtype)
    for g in range(2):
        a = u4[:, 2 * g]
        bm = u4[:, 2 * g + 1]
        if rev:
            a = a[:, :, ::-1]
            bm = bm[:, :, ::-1]
        swi[:, g, :, 0::2] = a
        swi[:, g, :, 1::2] = bm
    return np.ascontiguousarray(swi)


def make_in_maps(u, M):
    u8 = np.asarray(u).astype(FP8_NP)
    m8 = np.asarray(M).astype(FP8_NP)
    mat3 = _to_k_major(m8, N)
    in_maps = []
    for i in range(N_CORES):
        uT_i = np.ascontiguousarray(u8[i * SHARD:(i + 1) * SHARD, :].T)
        u3 = _to_k_major(uT_i, SHARD)
        in_maps.append({"uT": make_swi(u3, REV) if SWI else u3,
                        "mat": mat3})
    return in_maps


def kernel(u, crc_gen, info_pos, ind_gather, perm_out):
    from concourse.bass_utils import run_bass_kernel_spmd

    M = build_M(crc_gen, info_pos, ind_gather, perm_out)
    in_maps = make_in_maps(u, M)
    nc = get_nc()
    res = run_bass_kernel_spmd(nc, in_maps, core_ids=list(range(N_CORES)))
    out = np.concatenate(
        [np.asarray(r["y"]).astype(np.float32) for r in res.results], axis=0
    )
    return out


# revision 5
# speedup vs baseline: 1.0762x; 1.0165x over previous
"""5G Polar encoder on 8 trn2 cores: one fused GF(2) matmul,
DoubleRowSwInterleave weights.

The whole reference computation is GF(2)-linear in u, so the host
composes one binary matrix M [512, 1024] from the tiny index tables and
the device computes y = (u @ M) mod 2, data-parallel over the batch
(8192 rows/core), as an fp8e4 matmul accumulating in f32 PSUM (exact:
sums <= 512), with mod-2 on the eviction path (ACT/DVE f32->i16, DVE
AND 1) and i16 {0,1} DMA'd out (host converts to f32).

Matmul perf mode is DoubleRowSwInterleave: the stationary operand is
stored flat [p, 256] with the two DR k-row-sets interleaved per column
and columns REVERSED (bass_interp.py:5260):
    F[p, 2*(127-m)]   = W0[p, m]   (k-row-set A = ks)
    F[p, 2*(127-m)+1] = W1[p, m]   (k-row-set B = ks+1)
so the PE reads weights contiguously.  Measured: the matmul stage drops
from ~75us (plain DoubleRow, LDWEIGHTS exposes ~80ns/mm since DR
disables fast weight load) to ~60-62us; full kernel ~76us vs ~79us.
Matmul order alternates PSUM banks (same-bank RMW back-to-back stalls)
and the stationary serves both psum halves.  Eviction split ACT 44 /
DVE 20 tiles balances both engines at ~45us, hidden under PE.  i8
output (extra DVE narrow) measured WORSE (80us) - engine time, not
HBM, is the secondary constraint.
"""

import numpy as np
import ml_dtypes

N_CORES = 8
BS = 65536
K = 512
N = 1024
SHARD = BS // N_CORES
P = 128
KT = K // P
NB = SHARD // P

FP8_NP = ml_dtypes.float8_e4m3

_nc_cache = {}


def build_M(crc_gen, info_pos, ind_gather, perm_out):
    crc_gen = np.asarray(crc_gen)
    info_pos = np.asarray(info_pos)
    ind_gather = np.asarray(ind_gather)
    perm_out = np.asarray(perm_out)
    k, _ = crc_gen.shape
    nb, n1 = ind_gather.shape
    kp = info_pos.shape[0]
    C = (crc_gen.astype(np.int64) & 1).astype(np.uint8)
    B = np.concatenate([np.eye(k, dtype=np.uint8), C], axis=1)
    col_src = np.full(n1, -1, np.int64)
    col_src[info_pos] = np.arange(kp)
    A = np.zeros((k, n1), np.uint8)
    valid = col_src >= 0
    A[:, valid] = B[:, col_src[valid]]
    for s in range(nb):
        A = A ^ A[:, ind_gather[s]]
    return A[:, perm_out]


def _build_nc(reps=1, w1_act=48, u_chunks=4, wbufs=6, uload_in_reps=False,
              swi=True, rev=True, variant="full", evict8=False,
              lq_act=True):
    import concourse.tile as tile
    from concourse import bacc, mybir

    nc = bacc.Bacc("TRN2", target_bir_lowering=False, debug=False)
    fp8 = mybir.dt.float8e4
    f32 = mybir.dt.float32
    i16 = mybir.dt.int16
    i8 = mybir.dt.int8
    MODE = (mybir.MatmulPerfMode.DoubleRowSwInterleave if swi
            else mybir.MatmulPerfMode.DoubleRow)

    if swi:
        uT = nc.declare_dram_parameter("uT", [P, 2, NB, 2 * P], fp8,
                                       isOutput=False)
    else:
        uT = nc.declare_dram_parameter("uT", [P, KT, SHARD], fp8,
                                       isOutput=False)
    mat = nc.declare_dram_parameter("mat", [P, KT, N], fp8, isOutput=False)
    y = nc.declare_dram_parameter("y", [SHARD, N], i8 if evict8 else i16,
                                  isOutput=True)

    NBC = NB // u_chunks   # batch tiles per chunk (swi layout)
    CW = SHARD // u_chunks
    with tile.TileContext(nc) as tc:
        with (
            tc.tile_pool(name="consts", bufs=1) as cpool,
            tc.tile_pool(name="work", bufs=wbufs) as wpool,
            tc.tile_pool(name="outs", bufs=6) as opool,
            tc.tile_pool(name="psum", bufs=4, space="PSUM") as ppool,
        ):
            mt = cpool.tile([P, KT, N], fp8, tag="mt")
            if swi:
                uts = [
                    cpool.tile([P, 2, NBC, 2 * P], fp8, tag=f"ut{c}",
                               name=f"ut{c}")
                    for c in range(u_chunks)
                ]
            else:
                uts = [
                    cpool.tile([P, KT, CW], fp8, tag=f"ut{c}", name=f"ut{c}")
                    for c in range(u_chunks)
                ]
            for r in range(reps):
                if r == 0 or uload_in_reps:
                    # input loads issue on the ACT HW-DGE ring; output
                    # stores use the SP ring (HWDGE is FIFO per ring, so
                    # sharing one ring queues loads behind 64 y-stores)
                    ldq = nc.scalar if lq_act else nc.sync
                    ldq.dma_start(mt[:], mat[:])
                    for c in range(u_chunks):
                        if swi:
                            ldq.dma_start(
                                uts[c][:],
                                uT[:, :, c * NBC:(c + 1) * NBC, :])
                        else:
                            ldq.dma_start(
                                uts[c][:], uT[:, :, c * CW:(c + 1) * CW])
                for b in range(NB):
                    i = r * NB + b
                    ps = ppool.tile([P, N], f32, tag="ps", name="ps")
                    if swi:
                        ut = uts[b // NBC]
                        bl = b % NBC
                    else:
                        ut = uts[(b * P) // CW]
                        boff = (b * P) % CW
                    for ks in (0, 2):
                        g = ks // 2
                        for h in range(2):
                            lhsT = (ut[:, g, bl, :] if swi
                                    else ut[:, ks:ks + 2, boff:boff + P])
                            nc.tensor.matmul(
                                ps[:, h * 512:(h + 1) * 512],
                                lhsT,
                                mt[:, ks:ks + 2, h * 512:(h + 1) * 512],
                                start=(ks == 0),
                                stop=(ks == 2),
                                perf_mode=MODE,
                                skip_group_check=True,
                            )
                    if variant == "mm":
                        continue
                    t16 = wpool.tile([P, N], i16, tag="t16")
                    a16 = wpool.tile([P, N], i16, tag="a16")
                    # interleave the ACT/DVE W1 assignment evenly: a
                    # clustered split lets the busier engine fall behind
                    # the PE rate and stall psum recycling
                    if (i * w1_act) % NB < w1_act:
                        nc.scalar.activation(
                            t16[:], ps[:],
                            mybir.ActivationFunctionType.Copy,
                        )
                    else:
                        nc.vector.tensor_copy(t16[:], ps[:])
                    nc.vector.tensor_scalar(
                        a16[:], t16[:], 1, None,
                        mybir.AluOpType.bitwise_and,
                    )
                    if evict8:
                        o8 = opool.tile([P, N], i8, tag="o8")
                        nc.vector.tensor_copy(o8[:], a16[:])
                        nc.sync.dma_start(y[b * P:(b + 1) * P, :], o8[:])
                    else:
                        nc.sync.dma_start(y[b * P:(b + 1) * P, :], a16[:])
    nc.compile()
    return nc


W1_ACT = 44
U_CHUNKS = 4
WBUFS = 8
SWI = True
REV = True
EVICT8 = False
LQ_ACT = False


def get_nc(reps=1, uload_in_reps=False, variant="full"):
    key = (reps, W1_ACT, U_CHUNKS, WBUFS, uload_in_reps, SWI, REV, variant,
           EVICT8, LQ_ACT)
    if key not in _nc_cache:
        _nc_cache[key] = _build_nc(reps, w1_act=W1_ACT, u_chunks=U_CHUNKS,
                                   wbufs=WBUFS, uload_in_reps=uload_in_reps,
                                   swi=SWI, rev=REV, variant=variant,
                                   evict8=EVICT8, lq_act=LQ_ACT)
    return _nc_cache[key]


def _to_k_major(a_km, free):
    return np.ascontiguousarray(
        a_km.reshape(KT, P, free).transpose(1, 0, 2)
    )


def make_swi(u3, rev=True):
    """[P, KT, SHARD] -> [P, 2, NB, 256] SwInterleave stationary layout."""
    u4 = u3.reshape(P, KT, NB, P)
    swi = np.empty((P, 2, NB, 2 * P), u3.dtype)
    for g in range(2):
        a = u4[:, 2 * g]
        bm = u4[:, 2 * g + 1]
        if rev:
            a = a[:, :, ::-1]
            bm = bm[:, :, ::-1]
        swi[:, g, :, 0::2] = a
        swi[:, g, :, 1::2] = bm
    return np.ascontiguousarray(swi)


def make_in_maps(u, M):
    u8 = np.asarray(u).astype(FP8_NP)
    m8 = np.asarray(M).astype(FP8_NP)
    mat3 = _to_k_major(m8, N)
    in_maps = []
    for i in range(N_CORES):
        uT_i = np.ascontiguousarray(u8[i * SHARD:(i + 1) * SHARD, :].T)
        u3 = _to_k_major(uT_i, SHARD)
        in_maps.append({"uT": make_swi(u3, REV) if SWI else u3,
                        "mat": mat3})
    return in_maps


def kernel(u, crc_gen, info_pos, ind_gather, perm_out):
    from concourse.bass_utils import run_bass_kernel_spmd

    M = build_M(crc_gen, info_pos, ind_gather, perm_out)
    in_maps = make_in_maps(u, M)
    nc = get_nc()
    res = run_bass_kernel_spmd(nc, in_maps, core_ids=list(range(N_CORES)))
    out = np.concatenate(
        [np.asarray(r["y"]).astype(np.float32) for r in res.results], axis=0
    )
    return out


# revision 6
# speedup vs baseline: 1.0986x; 1.0209x over previous
"""5G Polar encoder on 8 trn2 cores: one fused GF(2) matmul,
DoubleRowSwInterleave weights.

The whole reference computation is GF(2)-linear in u, so the host
composes one binary matrix M [512, 1024] from the tiny index tables and
the device computes y = (u @ M) mod 2, data-parallel over the batch
(8192 rows/core), as an fp8e4 matmul accumulating in f32 PSUM (exact:
sums <= 512), with mod-2 on the eviction path (ACT/DVE f32->i16, DVE
AND 1) and i16 {0,1} DMA'd out (host converts to f32).

Matmul perf mode is DoubleRowSwInterleave: the stationary operand is
stored flat [p, 256] with the two DR k-row-sets interleaved per column
and columns REVERSED (bass_interp.py:5260):
    F[p, 2*(127-m)]   = W0[p, m]   (k-row-set A = ks)
    F[p, 2*(127-m)+1] = W1[p, m]   (k-row-set B = ks+1)
so the PE reads weights contiguously.  Measured: the matmul stage drops
from ~75us (plain DoubleRow, LDWEIGHTS exposes ~80ns/mm since DR
disables fast weight load) to ~60-62us; full kernel ~76us vs ~79us.
Matmul order alternates PSUM banks (same-bank RMW back-to-back stalls)
and the stationary serves both psum halves.  Eviction split ACT 44 /
DVE 20 tiles balances both engines at ~45us, hidden under PE.  i8
output (extra DVE narrow) measured WORSE (80us) - engine time, not
HBM, is the secondary constraint.
"""

import numpy as np
import ml_dtypes

N_CORES = 8
BS = 65536
K = 512
N = 1024
SHARD = BS // N_CORES
P = 128
KT = K // P
NB = SHARD // P

FP8_NP = ml_dtypes.float8_e4m3

_nc_cache = {}


def build_M(crc_gen, info_pos, ind_gather, perm_out):
    crc_gen = np.asarray(crc_gen)
    info_pos = np.asarray(info_pos)
    ind_gather = np.asarray(ind_gather)
    perm_out = np.asarray(perm_out)
    k, _ = crc_gen.shape
    nb, n1 = ind_gather.shape
    kp = info_pos.shape[0]
    C = (crc_gen.astype(np.int64) & 1).astype(np.uint8)
    B = np.concatenate([np.eye(k, dtype=np.uint8), C], axis=1)
    col_src = np.full(n1, -1, np.int64)
    col_src[info_pos] = np.arange(kp)
    A = np.zeros((k, n1), np.uint8)
    valid = col_src >= 0
    A[:, valid] = B[:, col_src[valid]]
    for s in range(nb):
        A = A ^ A[:, ind_gather[s]]
    return A[:, perm_out]


def _build_nc(reps=1, w1_act=48, u_chunks=4, wbufs=6, uload_in_reps=False,
              swi=True, rev=True, variant="full", evict8=False,
              lq_act=True):
    import concourse.tile as tile
    from concourse import bacc, mybir

    nc = bacc.Bacc("TRN2", target_bir_lowering=False, debug=False)
    fp8 = mybir.dt.float8e4
    f32 = mybir.dt.float32
    i16 = mybir.dt.int16
    i8 = mybir.dt.int8
    MODE = (mybir.MatmulPerfMode.DoubleRowSwInterleave if swi
            else mybir.MatmulPerfMode.DoubleRow)

    if swi:
        uT = nc.declare_dram_parameter("uT", [P, 2, NB, 2 * P], fp8,
                                       isOutput=False)
    else:
        uT = nc.declare_dram_parameter("uT", [P, KT, SHARD], fp8,
                                       isOutput=False)
    mat = nc.declare_dram_parameter("mat", [P, KT, N], fp8, isOutput=False)
    y = nc.declare_dram_parameter("y", [SHARD, N], i8 if evict8 else i16,
                                  isOutput=True)

    NBC = NB // u_chunks   # batch tiles per chunk (swi layout)
    CW = SHARD // u_chunks
    with tile.TileContext(nc) as tc:
        with (
            tc.tile_pool(name="consts", bufs=1) as cpool,
            tc.tile_pool(name="work", bufs=wbufs) as wpool,
            tc.tile_pool(name="outs", bufs=6) as opool,
            tc.tile_pool(name="psum", bufs=4, space="PSUM") as ppool,
        ):
            mt = cpool.tile([P, KT, N], fp8, tag="mt")
            if swi:
                uts = [
                    cpool.tile([P, 2, NBC, 2 * P], fp8, tag=f"ut{c}",
                               name=f"ut{c}")
                    for c in range(u_chunks)
                ]
            else:
                uts = [
                    cpool.tile([P, KT, CW], fp8, tag=f"ut{c}", name=f"ut{c}")
                    for c in range(u_chunks)
                ]
            for r in range(reps):
                if r == 0 or uload_in_reps:
                    # input loads issue on the ACT HW-DGE ring; output
                    # stores use the SP ring (HWDGE is FIFO per ring, so
                    # sharing one ring queues loads behind 64 y-stores)
                    ldq = nc.scalar if lq_act else nc.sync
                    ldq.dma_start(mt[:], mat[:])
                    for c in range(u_chunks):
                        if swi:
                            ldq.dma_start(
                                uts[c][:],
                                uT[:, :, c * NBC:(c + 1) * NBC, :])
                        else:
                            ldq.dma_start(
                                uts[c][:], uT[:, :, c * CW:(c + 1) * CW])
                for b in range(NB):
                    i = r * NB + b
                    ps = ppool.tile([P, N], f32, tag="ps", name="ps")
                    if swi:
                        ut = uts[b // NBC]
                        bl = b % NBC
                    else:
                        ut = uts[(b * P) // CW]
                        boff = (b * P) % CW
                    for ks in (0, 2):
                        g = ks // 2
                        for h in range(2):
                            lhsT = (ut[:, g, bl, :] if swi
                                    else ut[:, ks:ks + 2, boff:boff + P])
                            nc.tensor.matmul(
                                ps[:, h * 512:(h + 1) * 512],
                                lhsT,
                                mt[:, ks:ks + 2, h * 512:(h + 1) * 512],
                                start=(ks == 0),
                                stop=(ks == 2),
                                perf_mode=MODE,
                                skip_group_check=True,
                            )
                    if variant == "mm":
                        continue
                    t16 = wpool.tile([P, N], i16, tag="t16")
                    a16 = wpool.tile([P, N], i16, tag="a16")
                    # interleave the ACT/DVE W1 assignment evenly: a
                    # clustered split lets the busier engine fall behind
                    # the PE rate and stall psum recycling
                    if (i * w1_act) % NB < w1_act:
                        nc.scalar.activation(
                            t16[:], ps[:],
                            mybir.ActivationFunctionType.Copy,
                        )
                    else:
                        nc.vector.tensor_copy(t16[:], ps[:])
                    nc.vector.tensor_scalar(
                        a16[:], t16[:], 1, None,
                        mybir.AluOpType.bitwise_and,
                    )
                    if evict8:
                        o8 = opool.tile([P, N], i8, tag="o8")
                        nc.vector.tensor_copy(o8[:], a16[:])
                        nc.sync.dma_start(y[b * P:(b + 1) * P, :], o8[:])
                    else:
                        nc.sync.dma_start(y[b * P:(b + 1) * P, :], a16[:])
    nc.compile()
    return nc


W1_ACT = 44
U_CHUNKS = 4
WBUFS = 12
SWI = True
REV = True
EVICT8 = False
LQ_ACT = False


def get_nc(reps=1, uload_in_reps=False, variant="full"):
    key = (reps, W1_ACT, U_CHUNKS, WBUFS, uload_in_reps, SWI, REV, variant,
           EVICT8, LQ_ACT)
    if key not in _nc_cache:
        _nc_cache[key] = _build_nc(reps, w1_act=W1_ACT, u_chunks=U_CHUNKS,
                                   wbufs=WBUFS, uload_in_reps=uload_in_reps,
                                   swi=SWI, rev=REV, variant=variant,
                                   evict8=EVICT8, lq_act=LQ_ACT)
    return _nc_cache[key]


def _to_k_major(a_km, free):
    return np.ascontiguousarray(
        a_km.reshape(KT, P, free).transpose(1, 0, 2)
    )


def make_swi(u3, rev=True):
    """[P, KT, SHARD] -> [P, 2, NB, 256] SwInterleave stationary layout."""
    u4 = u3.reshape(P, KT, NB, P)
    swi = np.empty((P, 2, NB, 2 * P), u3.dtype)
    for g in range(2):
        a = u4[:, 2 * g]
        bm = u4[:, 2 * g + 1]
        if rev:
            a = a[:, :, ::-1]
            bm = bm[:, :, ::-1]
        swi[:, g, :, 0::2] = a
        swi[:, g, :, 1::2] = bm
    return np.ascontiguousarray(swi)


def make_in_maps(u, M):
    u8 = np.asarray(u).astype(FP8_NP)
    m8 = np.asarray(M).astype(FP8_NP)
    mat3 = _to_k_major(m8, N)
    in_maps = []
    for i in range(N_CORES):
        uT_i = np.ascontiguousarray(u8[i * SHARD:(i + 1) * SHARD, :].T)
        u3 = _to_k_major(uT_i, SHARD)
        in_maps.append({"uT": make_swi(u3, REV) if SWI else u3,
                        "mat": mat3})
    return in_maps


def kernel(u, crc_gen, info_pos, ind_gather, perm_out):
    from concourse.bass_utils import run_bass_kernel_spmd

    M = build_M(crc_gen, info_pos, ind_gather, perm_out)
    in_maps = make_in_maps(u, M)
    nc = get_nc()
    res = run_bass_kernel_spmd(nc, in_maps, core_ids=list(range(N_CORES)))
    out = np.concatenate(
        [np.asarray(r["y"]).astype(np.float32) for r in res.results], axis=0
    )
    return out


# revision 8
# speedup vs baseline: 1.1129x; 1.0130x over previous
"""5G Polar encoder on 8 trn2 cores: one fused GF(2) matmul,
DoubleRowSwInterleave weights.

The whole reference computation is GF(2)-linear in u, so the host
composes one binary matrix M [512, 1024] from the tiny index tables and
the device computes y = (u @ M) mod 2, data-parallel over the batch
(8192 rows/core), as an fp8e4 matmul accumulating in f32 PSUM (exact:
sums <= 512), with mod-2 on the eviction path (ACT/DVE f32->i16, DVE
AND 1) and i16 {0,1} DMA'd out (host converts to f32).

Matmul perf mode is DoubleRowSwInterleave: the stationary operand is
stored flat [p, 256] with the two DR k-row-sets interleaved per column
and columns REVERSED (bass_interp.py:5260):
    F[p, 2*(127-m)]   = W0[p, m]   (k-row-set A = ks)
    F[p, 2*(127-m)+1] = W1[p, m]   (k-row-set B = ks+1)
so the PE reads weights contiguously.  Measured: the matmul stage drops
from ~75us (plain DoubleRow, LDWEIGHTS exposes ~80ns/mm since DR
disables fast weight load) to ~60-62us; full kernel ~76us vs ~79us.
Matmul order alternates PSUM banks (same-bank RMW back-to-back stalls)
and the stationary serves both psum halves.  Eviction split ACT 44 /
DVE 20 tiles balances both engines at ~45us, hidden under PE.  i8
output (extra DVE narrow) measured WORSE (80us) - engine time, not
HBM, is the secondary constraint.
"""

import numpy as np
import ml_dtypes

N_CORES = 8
BS = 65536
K = 512
N = 1024
SHARD = BS // N_CORES
P = 128
KT = K // P
NB = SHARD // P

FP8_NP = ml_dtypes.float8_e4m3

_nc_cache = {}


def build_M(crc_gen, info_pos, ind_gather, perm_out):
    crc_gen = np.asarray(crc_gen)
    info_pos = np.asarray(info_pos)
    ind_gather = np.asarray(ind_gather)
    perm_out = np.asarray(perm_out)
    k, _ = crc_gen.shape
    nb, n1 = ind_gather.shape
    kp = info_pos.shape[0]
    C = (crc_gen.astype(np.int64) & 1).astype(np.uint8)
    B = np.concatenate([np.eye(k, dtype=np.uint8), C], axis=1)
    col_src = np.full(n1, -1, np.int64)
    col_src[info_pos] = np.arange(kp)
    A = np.zeros((k, n1), np.uint8)
    valid = col_src >= 0
    A[:, valid] = B[:, col_src[valid]]
    for s in range(nb):
        A = A ^ A[:, ind_gather[s]]
    return A[:, perm_out]


def _build_nc(reps=1, w1_act=48, u_chunks=4, wbufs=6, uload_in_reps=False,
              swi=True, rev=True, variant="full", evict8=False,
              lq_act=True, half_ev=False):
    import concourse.tile as tile
    from concourse import bacc, mybir

    nc = bacc.Bacc("TRN2", target_bir_lowering=False, debug=False)
    fp8 = mybir.dt.float8e4
    f32 = mybir.dt.float32
    i16 = mybir.dt.int16
    i8 = mybir.dt.int8
    MODE = (mybir.MatmulPerfMode.DoubleRowSwInterleave if swi
            else mybir.MatmulPerfMode.DoubleRow)

    if swi:
        uT = nc.declare_dram_parameter("uT", [P, 2, NB, 2 * P], fp8,
                                       isOutput=False)
    else:
        uT = nc.declare_dram_parameter("uT", [P, KT, SHARD], fp8,
                                       isOutput=False)
    mat = nc.declare_dram_parameter("mat", [P, KT, N], fp8, isOutput=False)
    y = nc.declare_dram_parameter("y", [SHARD, N], i8 if evict8 else i16,
                                  isOutput=True)

    NBC = NB // u_chunks   # batch tiles per chunk (swi layout)
    CW = SHARD // u_chunks
    with tile.TileContext(nc) as tc:
        with (
            tc.tile_pool(name="consts", bufs=1) as cpool,
            tc.tile_pool(name="work", bufs=wbufs) as wpool,
            tc.tile_pool(name="outs", bufs=6) as opool,
            tc.tile_pool(name="psum", bufs=8 if half_ev else 4,
                         space="PSUM") as ppool,
        ):
            mt = cpool.tile([P, KT, N], fp8, tag="mt")
            if swi:
                uts = [
                    cpool.tile([P, 2, NBC, 2 * P], fp8, tag=f"ut{c}",
                               name=f"ut{c}")
                    for c in range(u_chunks)
                ]
            else:
                uts = [
                    cpool.tile([P, KT, CW], fp8, tag=f"ut{c}", name=f"ut{c}")
                    for c in range(u_chunks)
                ]
            for r in range(reps):
                if r == 0 or uload_in_reps:
                    # input loads issue on the ACT HW-DGE ring; output
                    # stores use the SP ring (HWDGE is FIFO per ring, so
                    # sharing one ring queues loads behind 64 y-stores)
                    ldq = nc.scalar if lq_act else nc.sync
                    ldq.dma_start(mt[:], mat[:])
                    for c in range(u_chunks):
                        if swi:
                            ldq.dma_start(
                                uts[c][:],
                                uT[:, :, c * NBC:(c + 1) * NBC, :])
                        else:
                            ldq.dma_start(
                                uts[c][:], uT[:, :, c * CW:(c + 1) * CW])
                for b in range(NB):
                    i = r * NB + b
                    if half_ev:
                        pss = [ppool.tile([P, 512], f32, tag="ps",
                                          name=f"ps{h}") for h in range(2)]
                    else:
                        ps = ppool.tile([P, N], f32, tag="ps", name="ps")
                    if swi:
                        ut = uts[b // NBC]
                        bl = b % NBC
                    else:
                        ut = uts[(b * P) // CW]
                        boff = (b * P) % CW
                    for ks in (0, 2):
                        g = ks // 2
                        for h in range(2):
                            lhsT = (ut[:, g, bl, :] if swi
                                    else ut[:, ks:ks + 2, boff:boff + P])
                            out_ap = (pss[h][:] if half_ev
                                      else ps[:, h * 512:(h + 1) * 512])
                            nc.tensor.matmul(
                                out_ap,
                                lhsT,
                                mt[:, ks:ks + 2, h * 512:(h + 1) * 512],
                                start=(ks == 0),
                                stop=(ks == 2),
                                perf_mode=MODE,
                                skip_group_check=True,
                            )
                    if variant == "mm":
                        continue
                    t16 = wpool.tile([P, N], i16, tag="t16")
                    a16 = wpool.tile([P, N], i16, tag="a16")
                    if half_ev:
                        # per-half W1: each [P,512] psum bank is freed by
                        # its own op; ~2/3 of halves on ACT, rest DVE,
                        # spread evenly
                        for h in range(2):
                            j = 2 * i + h
                            if (j * 86) % 128 < 86:
                                nc.scalar.activation(
                                    t16[:, h * 512:(h + 1) * 512],
                                    pss[h][:],
                                    mybir.ActivationFunctionType.Copy,
                                )
                            else:
                                nc.vector.tensor_copy(
                                    t16[:, h * 512:(h + 1) * 512],
                                    pss[h][:],
                                )
                    # interleave the ACT/DVE W1 assignment evenly: a
                    # clustered split lets the busier engine fall behind
                    # the PE rate and stall psum recycling
                    elif (i * w1_act) % NB < w1_act:
                        nc.scalar.activation(
                            t16[:], ps[:],
                            mybir.ActivationFunctionType.Copy,
                        )
                    else:
                        nc.vector.tensor_copy(t16[:], ps[:])
                    nc.vector.tensor_scalar(
                        a16[:], t16[:], 1, None,
                        mybir.AluOpType.bitwise_and,
                    )
                    if evict8:
                        o8 = opool.tile([P, N], i8, tag="o8")
                        nc.vector.tensor_copy(o8[:], a16[:])
                        nc.sync.dma_start(y[b * P:(b + 1) * P, :], o8[:])
                    else:
                        nc.sync.dma_start(y[b * P:(b + 1) * P, :], a16[:])
    nc.compile()
    return nc


W1_ACT = 44
U_CHUNKS = 4
WBUFS = 12
SWI = True
REV = True
EVICT8 = False
LQ_ACT = False
HALF_EV = False


def get_nc(reps=1, uload_in_reps=False, variant="full"):
    key = (reps, W1_ACT, U_CHUNKS, WBUFS, uload_in_reps, SWI, REV, variant,
           EVICT8, LQ_ACT, HALF_EV)
    if key not in _nc_cache:
        _nc_cache[key] = _build_nc(reps, w1_act=W1_ACT, u_chunks=U_CHUNKS,
                                   wbufs=WBUFS, uload_in_reps=uload_in_reps,
                                   swi=SWI, rev=REV, variant=variant,
                                   evict8=EVICT8, lq_act=LQ_ACT,
                                   half_ev=HALF_EV)
    return _nc_cache[key]


def _to_k_major(a_km, free):
    return np.ascontiguousarray(
        a_km.reshape(KT, P, free).transpose(1, 0, 2)
    )


def make_swi(u3, rev=True):
    """[P, KT, SHARD] -> [P, 2, NB, 256] SwInterleave stationary layout."""
    u4 = u3.reshape(P, KT, NB, P)
    swi = np.empty((P, 2, NB, 2 * P), u3.dtype)
    for g in range(2):
        a = u4[:, 2 * g]
        bm = u4[:, 2 * g + 1]
        if rev:
            a = a[:, :, ::-1]
            bm = bm[:, :, ::-1]
        swi[:, g, :, 0::2] = a
        swi[:, g, :, 1::2] = bm
    return np.ascontiguousarray(swi)


def make_in_maps(u, M):
    u8 = np.asarray(u).astype(FP8_NP)
    m8 = np.asarray(M).astype(FP8_NP)
    mat3 = _to_k_major(m8, N)
    in_maps = []
    for i in range(N_CORES):
        uT_i = np.ascontiguousarray(u8[i * SHARD:(i + 1) * SHARD, :].T)
        u3 = _to_k_major(uT_i, SHARD)
        in_maps.append({"uT": make_swi(u3, REV) if SWI else u3,
                        "mat": mat3})
    return in_maps


def kernel(u, crc_gen, info_pos, ind_gather, perm_out):
    from concourse.bass_utils import run_bass_kernel_spmd

    M = build_M(crc_gen, info_pos, ind_gather, perm_out)
    in_maps = make_in_maps(u, M)
    nc = get_nc()
    res = run_bass_kernel_spmd(nc, in_maps, core_ids=list(range(N_CORES)))
    out = np.concatenate(
        [np.asarray(r["y"]).astype(np.float32) for r in res.results], axis=0
    )
    return out


# revision 10
# speedup vs baseline: 1.1580x; 1.0405x over previous
"""5G Polar encoder on 8 trn2 cores: one fused GF(2) matmul,
DoubleRowSwInterleave weights.

The whole reference computation is GF(2)-linear in u, so the host
composes one binary matrix M [512, 1024] from the tiny index tables and
the device computes y = (u @ M) mod 2, data-parallel over the batch
(8192 rows/core), as an fp8e4 matmul accumulating in f32 PSUM (exact:
sums <= 512), with mod-2 on the eviction path (ACT/DVE f32->i16, DVE
AND 1) and i16 {0,1} DMA'd out (host converts to f32).

Matmul perf mode is DoubleRowSwInterleave: the stationary operand is
stored flat [p, 256] with the two DR k-row-sets interleaved per column
and columns REVERSED (bass_interp.py:5260):
    F[p, 2*(127-m)]   = W0[p, m]   (k-row-set A = ks)
    F[p, 2*(127-m)+1] = W1[p, m]   (k-row-set B = ks+1)
so the PE reads weights contiguously.  Measured: the matmul stage drops
from ~75us (plain DoubleRow, LDWEIGHTS exposes ~80ns/mm since DR
disables fast weight load) to ~60-62us; full kernel ~76us vs ~79us.
Matmul order alternates PSUM banks (same-bank RMW back-to-back stalls)
and the stationary serves both psum halves.  Eviction split ACT 44 /
DVE 20 tiles balances both engines at ~45us, hidden under PE.  i8
output (extra DVE narrow) measured WORSE (80us) - engine time, not
HBM, is the secondary constraint.
"""

import numpy as np
import ml_dtypes

N_CORES = 8
BS = 65536
K = 512
N = 1024
SHARD = BS // N_CORES
P = 128
KT = K // P
NB = SHARD // P

FP8_NP = ml_dtypes.float8_e4m3

_nc_cache = {}


def build_M(crc_gen, info_pos, ind_gather, perm_out):
    crc_gen = np.asarray(crc_gen)
    info_pos = np.asarray(info_pos)
    ind_gather = np.asarray(ind_gather)
    perm_out = np.asarray(perm_out)
    k, _ = crc_gen.shape
    nb, n1 = ind_gather.shape
    kp = info_pos.shape[0]
    C = (crc_gen.astype(np.int64) & 1).astype(np.uint8)
    B = np.concatenate([np.eye(k, dtype=np.uint8), C], axis=1)
    col_src = np.full(n1, -1, np.int64)
    col_src[info_pos] = np.arange(kp)
    A = np.zeros((k, n1), np.uint8)
    valid = col_src >= 0
    A[:, valid] = B[:, col_src[valid]]
    for s in range(nb):
        A = A ^ A[:, ind_gather[s]]
    return A[:, perm_out]


def _build_nc(reps=1, w1_act=48, u_chunks=4, wbufs=6, uload_in_reps=False,
              swi=True, rev=True, variant="full", evict8=False,
              lq_act=True, half_ev=False):
    import concourse.tile as tile
    from concourse import bacc, mybir

    nc = bacc.Bacc("TRN2", target_bir_lowering=False, debug=False)
    fp8 = mybir.dt.float8e4
    f32 = mybir.dt.float32
    i16 = mybir.dt.int16
    i8 = mybir.dt.int8
    MODE = (mybir.MatmulPerfMode.DoubleRowSwInterleave if swi
            else mybir.MatmulPerfMode.DoubleRow)

    if swi:
        uT = nc.declare_dram_parameter("uT", [P, 2, NB, 2 * P], fp8,
                                       isOutput=False)
    else:
        uT = nc.declare_dram_parameter("uT", [P, KT, SHARD], fp8,
                                       isOutput=False)
    mat = nc.declare_dram_parameter("mat", [P, KT, N], fp8, isOutput=False)
    y = nc.declare_dram_parameter("y", [SHARD, N], i8 if evict8 else i16,
                                  isOutput=True)

    NBC = NB // u_chunks   # batch tiles per chunk (swi layout)
    CW = SHARD // u_chunks
    with tile.TileContext(nc) as tc:
        with (
            tc.tile_pool(name="consts", bufs=1) as cpool,
            tc.tile_pool(name="work", bufs=wbufs) as wpool,
            tc.tile_pool(name="outs", bufs=6) as opool,
            tc.tile_pool(name="psum", bufs=8 if half_ev else 4,
                         space="PSUM") as ppool,
        ):
            mt = cpool.tile([P, KT, N], fp8, tag="mt")
            if swi:
                uts = [
                    cpool.tile([P, 2, NBC, 2 * P], fp8, tag=f"ut{c}",
                               name=f"ut{c}")
                    for c in range(u_chunks)
                ]
            else:
                uts = [
                    cpool.tile([P, KT, CW], fp8, tag=f"ut{c}", name=f"ut{c}")
                    for c in range(u_chunks)
                ]
            for r in range(reps):
                if r == 0 or uload_in_reps:
                    # input loads issue on the idle gpsimd engine's DMA
                    # queue: HWDGE is FIFO per ring, so sharing the SP
                    # ring queues loads behind 64 y-stores (~15us exposed
                    # load); the ACT ring stalls W1s behind the load's
                    # WAR semaphore; gpsimd's stream is free to block
                    ldq = nc.gpsimd if lq_act else nc.sync
                    ldq.dma_start(mt[:], mat[:])
                    for c in range(u_chunks):
                        if swi:
                            ldq.dma_start(
                                uts[c][:],
                                uT[:, :, c * NBC:(c + 1) * NBC, :])
                        else:
                            ldq.dma_start(
                                uts[c][:], uT[:, :, c * CW:(c + 1) * CW])
                for b in range(NB):
                    i = r * NB + b
                    if half_ev:
                        pss = [ppool.tile([P, 512], f32, tag="ps",
                                          name=f"ps{h}") for h in range(2)]
                    else:
                        ps = ppool.tile([P, N], f32, tag="ps", name="ps")
                    if swi:
                        ut = uts[b // NBC]
                        bl = b % NBC
                    else:
                        ut = uts[(b * P) // CW]
                        boff = (b * P) % CW
                    for ks in (0, 2):
                        g = ks // 2
                        for h in range(2):
                            lhsT = (ut[:, g, bl, :] if swi
                                    else ut[:, ks:ks + 2, boff:boff + P])
                            out_ap = (pss[h][:] if half_ev
                                      else ps[:, h * 512:(h + 1) * 512])
                            nc.tensor.matmul(
                                out_ap,
                                lhsT,
                                mt[:, ks:ks + 2, h * 512:(h + 1) * 512],
                                start=(ks == 0),
                                stop=(ks == 2),
                                perf_mode=MODE,
                                skip_group_check=True,
                            )
                    if variant == "mm":
                        continue
                    t16 = wpool.tile([P, N], i16, tag="t16")
                    a16 = wpool.tile([P, N], i16, tag="a16")
                    if half_ev:
                        # per-half W1: each [P,512] psum bank is freed by
                        # its own op; ~2/3 of halves on ACT, rest DVE,
                        # spread evenly
                        for h in range(2):
                            j = 2 * i + h
                            if (j * 86) % 128 < 86:
                                nc.scalar.activation(
                                    t16[:, h * 512:(h + 1) * 512],
                                    pss[h][:],
                                    mybir.ActivationFunctionType.Copy,
                                )
                            else:
                                nc.vector.tensor_copy(
                                    t16[:, h * 512:(h + 1) * 512],
                                    pss[h][:],
                                )
                    # interleave the ACT/DVE W1 assignment evenly: a
                    # clustered split lets the busier engine fall behind
                    # the PE rate and stall psum recycling
                    elif (i * w1_act) % NB < w1_act:
                        nc.scalar.activation(
                            t16[:], ps[:],
                            mybir.ActivationFunctionType.Copy,
                        )
                    else:
                        nc.vector.tensor_copy(t16[:], ps[:])
                    nc.vector.tensor_scalar(
                        a16[:], t16[:], 1, None,
                        mybir.AluOpType.bitwise_and,
                    )
                    if evict8:
                        o8 = opool.tile([P, N], i8, tag="o8")
                        nc.vector.tensor_copy(o8[:], a16[:])
                        nc.sync.dma_start(y[b * P:(b + 1) * P, :], o8[:])
                    else:
                        nc.sync.dma_start(y[b * P:(b + 1) * P, :], a16[:])
    nc.compile()
    return nc


W1_ACT = 44
U_CHUNKS = 4
WBUFS = 12
SWI = True
REV = True
EVICT8 = False
LQ_ACT = True
HALF_EV = False


def get_nc(reps=1, uload_in_reps=False, variant="full"):
    key = (reps, W1_ACT, U_CHUNKS, WBUFS, uload_in_reps, SWI, REV, variant,
           EVICT8, LQ_ACT, HALF_EV)
    if key not in _nc_cache:
        _nc_cache[key] = _build_nc(reps, w1_act=W1_ACT, u_chunks=U_CHUNKS,
                                   wbufs=WBUFS, uload_in_reps=uload_in_reps,
                                   swi=SWI, rev=REV, variant=variant,
                                   evict8=EVICT8, lq_act=LQ_ACT,
                                   half_ev=HALF_EV)
    return _nc_cache[key]


def _to_k_major(a_km, free):
    return np.ascontiguousarray(
        a_km.reshape(KT, P, free).transpose(1, 0, 2)
    )


def make_swi(u3, rev=True):
    """[P, KT, SHARD] -> [P, 2, NB, 256] SwInterleave stationary layout."""
    u4 = u3.reshape(P, KT, NB, P)
    swi = np.empty((P, 2, NB, 2 * P), u3.dtype)
    for g in range(2):
        a = u4[:, 2 * g]
        bm = u4[:, 2 * g + 1]
        if rev:
            a = a[:, :, ::-1]
            bm = bm[:, :, ::-1]
        swi[:, g, :, 0::2] = a
        swi[:, g, :, 1::2] = bm
    return np.ascontiguousarray(swi)


def make_in_maps(u, M):
    u8 = np.asarray(u).astype(FP8_NP)
    m8 = np.asarray(M).astype(FP8_NP)
    mat3 = _to_k_major(m8, N)
    in_maps = []
    for i in range(N_CORES):
        uT_i = np.ascontiguousarray(u8[i * SHARD:(i + 1) * SHARD, :].T)
        u3 = _to_k_major(uT_i, SHARD)
        in_maps.append({"uT": make_swi(u3, REV) if SWI else u3,
                        "mat": mat3})
    return in_maps


def kernel(u, crc_gen, info_pos, ind_gather, perm_out):
    from concourse.bass_utils import run_bass_kernel_spmd

    M = build_M(crc_gen, info_pos, ind_gather, perm_out)
    in_maps = make_in_maps(u, M)
    nc = get_nc()
    res = run_bass_kernel_spmd(nc, in_maps, core_ids=list(range(N_CORES)))
    out = np.concatenate(
        [np.asarray(r["y"]).astype(np.float32) for r in res.results], axis=0
    )
    return out


# revision 13
# speedup vs baseline: 1.2380x; 1.0691x over previous
"""5G Polar encoder on 8 trn2 cores: one fused GF(2) matmul,
DoubleRowSwInterleave weights.

The whole reference computation is GF(2)-linear in u, so the host
composes one binary matrix M [512, 1024] from the tiny index tables and
the device computes y = (u @ M) mod 2, data-parallel over the batch
(8192 rows/core), as an fp8e4 matmul accumulating in f32 PSUM (exact:
sums <= 512), with mod-2 on the eviction path (ACT/DVE f32->i16, DVE
AND 1) and i16 {0,1} DMA'd out (host converts to f32).

Matmul perf mode is DoubleRowSwInterleave: the stationary operand is
stored flat [p, 256] with the two DR k-row-sets interleaved per column
and columns REVERSED (bass_interp.py:5260):
    F[p, 2*(127-m)]   = W0[p, m]   (k-row-set A = ks)
    F[p, 2*(127-m)+1] = W1[p, m]   (k-row-set B = ks+1)
so the PE reads weights contiguously.  Measured: the matmul stage drops
from ~75us (plain DoubleRow, LDWEIGHTS exposes ~80ns/mm since DR
disables fast weight load) to ~60-62us; full kernel ~76us vs ~79us.
Matmul order alternates PSUM banks (same-bank RMW back-to-back stalls)
and the stationary serves both psum halves.  Eviction split ACT 44 /
DVE 20 tiles balances both engines at ~45us, hidden under PE.  i8
output (extra DVE narrow) measured WORSE (80us) - engine time, not
HBM, is the secondary constraint.
"""

import numpy as np
import ml_dtypes

N_CORES = 8
BS = 65536
K = 512
N = 1024
SHARD = BS // N_CORES
P = 128
KT = K // P
NB = SHARD // P

FP8_NP = ml_dtypes.float8_e4m3

_nc_cache = {}


def build_M(crc_gen, info_pos, ind_gather, perm_out):
    crc_gen = np.asarray(crc_gen)
    info_pos = np.asarray(info_pos)
    ind_gather = np.asarray(ind_gather)
    perm_out = np.asarray(perm_out)
    k, _ = crc_gen.shape
    nb, n1 = ind_gather.shape
    kp = info_pos.shape[0]
    C = (crc_gen.astype(np.int64) & 1).astype(np.uint8)
    B = np.concatenate([np.eye(k, dtype=np.uint8), C], axis=1)
    col_src = np.full(n1, -1, np.int64)
    col_src[info_pos] = np.arange(kp)
    A = np.zeros((k, n1), np.uint8)
    valid = col_src >= 0
    A[:, valid] = B[:, col_src[valid]]
    for s in range(nb):
        A = A ^ A[:, ind_gather[s]]
    return A[:, perm_out]


def _build_nc(reps=1, w1_act=48, u_chunks=4, wbufs=6, uload_in_reps=False,
              swi=True, rev=True, variant="full", evict8=False,
              lq_act=True, half_ev=False):
    import concourse.tile as tile
    from concourse import bacc, mybir

    nc = bacc.Bacc("TRN2", target_bir_lowering=False, debug=False)
    fp8 = mybir.dt.float8e4
    f32 = mybir.dt.float32
    i16 = mybir.dt.int16
    i8 = mybir.dt.int8
    MODE = (mybir.MatmulPerfMode.DoubleRowSwInterleave if swi
            else mybir.MatmulPerfMode.DoubleRow)

    if swi:
        uT = nc.declare_dram_parameter("uT", [P, 2, NB, 2 * P], fp8,
                                       isOutput=False)
    else:
        uT = nc.declare_dram_parameter("uT", [P, KT, SHARD], fp8,
                                       isOutput=False)
    mat = nc.declare_dram_parameter("mat", [P, KT, N], fp8, isOutput=False)
    y = nc.declare_dram_parameter("y", [SHARD, N], i8 if evict8 else i16,
                                  isOutput=True)

    NBC = NB // u_chunks   # batch tiles per chunk (swi layout)
    CW = SHARD // u_chunks
    with tile.TileContext(nc) as tc:
        with (
            tc.tile_pool(name="consts", bufs=1) as cpool,
            tc.tile_pool(name="work", bufs=wbufs) as wpool,
            tc.tile_pool(name="outs", bufs=6) as opool,
            tc.tile_pool(name="psum", bufs=8 if half_ev else 4,
                         space="PSUM") as ppool,
        ):
            # double-buffered inputs: rep r uses buffer set r%2, so a
            # rep's reloads WAR-wait on reads from TWO reps ago (a full
            # rep of slack) instead of serializing at the rep boundary
            # on the previous rep's last reader (mt is read by the very
            # last matmul)
            mts = [cpool.tile([P, KT, N], fp8, tag=f"mt{d}", name=f"mt{d}")
                   for d in range(2)]
            if swi:
                utss = [[
                    cpool.tile([P, 2, NBC, 2 * P], fp8, tag=f"ut{d}_{c}",
                               name=f"ut{d}_{c}")
                    for c in range(u_chunks)] for d in range(2)]
            else:
                utss = [[
                    cpool.tile([P, KT, CW], fp8, tag=f"ut{d}_{c}",
                               name=f"ut{d}_{c}")
                    for c in range(u_chunks)] for d in range(2)]
            for r in range(reps):
                d = (r % 2) if uload_in_reps else 0
                mt = mts[d]
                uts = utss[d]
                if r == 0 or uload_in_reps:
                    # input loads issue on the idle gpsimd engine's DMA
                    # queue: HWDGE is FIFO per ring, so sharing the SP
                    # ring queues loads behind 64 y-stores (~15us exposed
                    # load); the ACT ring stalls W1s behind the load's
                    # WAR semaphore; gpsimd's stream is free to block
                    ldq = nc.gpsimd if lq_act else nc.sync
                    ldq.dma_start(mt[:], mat[:])
                    for c in range(u_chunks):
                        if swi:
                            ldq.dma_start(
                                uts[c][:],
                                uT[:, :, c * NBC:(c + 1) * NBC, :])
                        else:
                            ldq.dma_start(
                                uts[c][:], uT[:, :, c * CW:(c + 1) * CW])
                for b in range(NB):
                    i = r * NB + b
                    if half_ev:
                        pss = [ppool.tile([P, 512], f32, tag="ps",
                                          name=f"ps{h}") for h in range(2)]
                    else:
                        ps = ppool.tile([P, N], f32, tag="ps", name="ps")
                    if swi:
                        ut = uts[b // NBC]
                        bl = b % NBC
                    else:
                        ut = uts[(b * P) // CW]
                        boff = (b * P) % CW
                    for ks in (0, 2):
                        g = ks // 2
                        for h in range(2):
                            lhsT = (ut[:, g, bl, :] if swi
                                    else ut[:, ks:ks + 2, boff:boff + P])
                            out_ap = (pss[h][:] if half_ev
                                      else ps[:, h * 512:(h + 1) * 512])
                            nc.tensor.matmul(
                                out_ap,
                                lhsT,
                                mt[:, ks:ks + 2, h * 512:(h + 1) * 512],
                                start=(ks == 0),
                                stop=(ks == 2),
                                perf_mode=MODE,
                                skip_group_check=True,
                            )
                    if variant == "mm":
                        continue
                    t16 = wpool.tile([P, N], i16, tag="t16")
                    a16 = wpool.tile([P, N], i16, tag="a16")
                    if half_ev:
                        # per-half W1: each [P,512] psum bank is freed by
                        # its own op; ~2/3 of halves on ACT, rest DVE,
                        # spread evenly
                        for h in range(2):
                            j = 2 * i + h
                            if (j * 86) % 128 < 86:
                                nc.scalar.activation(
                                    t16[:, h * 512:(h + 1) * 512],
                                    pss[h][:],
                                    mybir.ActivationFunctionType.Copy,
                                )
                            else:
                                nc.vector.tensor_copy(
                                    t16[:, h * 512:(h + 1) * 512],
                                    pss[h][:],
                                )
                    # interleave the ACT/DVE W1 assignment evenly: a
                    # clustered split lets the busier engine fall behind
                    # the PE rate and stall psum recycling
                    elif (i * w1_act) % NB < w1_act:
                        nc.scalar.activation(
                            t16[:], ps[:],
                            mybir.ActivationFunctionType.Copy,
                        )
                    else:
                        nc.vector.tensor_copy(t16[:], ps[:])
                    nc.vector.tensor_scalar(
                        a16[:], t16[:], 1, None,
                        mybir.AluOpType.bitwise_and,
                    )
                    if evict8:
                        o8 = opool.tile([P, N], i8, tag="o8")
                        nc.vector.tensor_copy(o8[:], a16[:])
                        nc.sync.dma_start(y[b * P:(b + 1) * P, :], o8[:])
                    else:
                        nc.sync.dma_start(y[b * P:(b + 1) * P, :], a16[:])
    nc.compile()
    return nc


W1_ACT = 44
U_CHUNKS = 4
WBUFS = 12
SWI = True
REV = True
EVICT8 = False
LQ_ACT = True
HALF_EV = False


def get_nc(reps=1, uload_in_reps=False, variant="full"):
    key = (reps, W1_ACT, U_CHUNKS, WBUFS, uload_in_reps, SWI, REV, variant,
           EVICT8, LQ_ACT, HALF_EV)
    if key not in _nc_cache:
        _nc_cache[key] = _build_nc(reps, w1_act=W1_ACT, u_chunks=U_CHUNKS,
                                   wbufs=WBUFS, uload_in_reps=uload_in_reps,
                                   swi=SWI, rev=REV, variant=variant,
                                   evict8=EVICT8, lq_act=LQ_ACT,
                                   half_ev=HALF_EV)
    return _nc_cache[key]


def _to_k_major(a_km, free):
    return np.ascontiguousarray(
        a_km.reshape(KT, P, free).transpose(1, 0, 2)
    )


def make_swi(u3, rev=True):
    """[P, KT, SHARD] -> [P, 2, NB, 256] SwInterleave stationary layout."""
    u4 = u3.reshape(P, KT, NB, P)
    swi = np.empty((P, 2, NB, 2 * P), u3.dtype)
    for g in range(2):
        a = u4[:, 2 * g]
        bm = u4[:, 2 * g + 1]
        if rev:
            a = a[:, :, ::-1]
            bm = bm[:, :, ::-1]
        swi[:, g, :, 0::2] = a
        swi[:, g, :, 1::2] = bm
    return np.ascontiguousarray(swi)


def make_in_maps(u, M):
    u8 = np.asarray(u).astype(FP8_NP)
    m8 = np.asarray(M).astype(FP8_NP)
    mat3 = _to_k_major(m8, N)
    in_maps = []
    for i in range(N_CORES):
        uT_i = np.ascontiguousarray(u8[i * SHARD:(i + 1) * SHARD, :].T)
        u3 = _to_k_major(uT_i, SHARD)
        in_maps.append({"uT": make_swi(u3, REV) if SWI else u3,
                        "mat": mat3})
    return in_maps


def kernel(u, crc_gen, info_pos, ind_gather, perm_out):
    from concourse.bass_utils import run_bass_kernel_spmd

    M = build_M(crc_gen, info_pos, ind_gather, perm_out)
    in_maps = make_in_maps(u, M)
    nc = get_nc()
    res = run_bass_kernel_spmd(nc, in_maps, core_ids=list(range(N_CORES)))
    out = np.concatenate(
        [np.asarray(r["y"]).astype(np.float32) for r in res.results], axis=0
    )
    return out


# revision 15
# speedup vs baseline: 1.2487x; 1.0087x over previous
"""5G Polar encoder on 8 trn2 cores: one fused GF(2) matmul,
DoubleRowSwInterleave weights.

The whole reference computation is GF(2)-linear in u, so the host
composes one binary matrix M [512, 1024] from the tiny index tables and
the device computes y = (u @ M) mod 2, data-parallel over the batch
(8192 rows/core), as an fp8e4 matmul accumulating in f32 PSUM (exact:
sums <= 512), with mod-2 on the eviction path (ACT/DVE f32->i16, DVE
AND 1) and i16 {0,1} DMA'd out (host converts to f32).

Matmul perf mode is DoubleRowSwInterleave: the stationary operand is
stored flat [p, 256] with the two DR k-row-sets interleaved per column
and columns REVERSED (bass_interp.py:5260):
    F[p, 2*(127-m)]   = W0[p, m]   (k-row-set A = ks)
    F[p, 2*(127-m)+1] = W1[p, m]   (k-row-set B = ks+1)
so the PE reads weights contiguously.  Measured: the matmul stage drops
from ~75us (plain DoubleRow, LDWEIGHTS exposes ~80ns/mm since DR
disables fast weight load) to ~60-62us; full kernel ~76us vs ~79us.
Matmul order alternates PSUM banks (same-bank RMW back-to-back stalls)
and the stationary serves both psum halves.  Eviction split ACT 44 /
DVE 20 tiles balances both engines at ~45us, hidden under PE.  i8
output (extra DVE narrow) measured WORSE (80us) - engine time, not
HBM, is the secondary constraint.
"""

import numpy as np
import ml_dtypes

N_CORES = 8
BS = 65536
K = 512
N = 1024
SHARD = BS // N_CORES
P = 128
KT = K // P
NB = SHARD // P

FP8_NP = ml_dtypes.float8_e4m3

_nc_cache = {}


def build_M(crc_gen, info_pos, ind_gather, perm_out):
    crc_gen = np.asarray(crc_gen)
    info_pos = np.asarray(info_pos)
    ind_gather = np.asarray(ind_gather)
    perm_out = np.asarray(perm_out)
    k, _ = crc_gen.shape
    nb, n1 = ind_gather.shape
    kp = info_pos.shape[0]
    C = (crc_gen.astype(np.int64) & 1).astype(np.uint8)
    B = np.concatenate([np.eye(k, dtype=np.uint8), C], axis=1)
    col_src = np.full(n1, -1, np.int64)
    col_src[info_pos] = np.arange(kp)
    A = np.zeros((k, n1), np.uint8)
    valid = col_src >= 0
    A[:, valid] = B[:, col_src[valid]]
    for s in range(nb):
        A = A ^ A[:, ind_gather[s]]
    return A[:, perm_out]


def _build_nc(reps=1, w1_act=48, u_chunks=4, wbufs=6, uload_in_reps=False,
              swi=True, rev=True, variant="full", evict8=False,
              lq_act=True, half_ev=False):
    import concourse.tile as tile
    from concourse import bacc, mybir

    nc = bacc.Bacc("TRN2", target_bir_lowering=False, debug=False)
    fp8 = mybir.dt.float8e4
    f32 = mybir.dt.float32
    i16 = mybir.dt.int16
    i8 = mybir.dt.int8
    MODE = (mybir.MatmulPerfMode.DoubleRowSwInterleave if swi
            else mybir.MatmulPerfMode.DoubleRow)

    if swi:
        uT = nc.declare_dram_parameter("uT", [P, 2, NB, 2 * P], fp8,
                                       isOutput=False)
    else:
        uT = nc.declare_dram_parameter("uT", [P, KT, SHARD], fp8,
                                       isOutput=False)
    mat = nc.declare_dram_parameter("mat", [P, KT, N], fp8, isOutput=False)
    y = nc.declare_dram_parameter("y", [SHARD, N], i8 if evict8 else i16,
                                  isOutput=True)

    NBC = NB // u_chunks   # batch tiles per chunk (swi layout)
    CW = SHARD // u_chunks
    with tile.TileContext(nc) as tc:
        with (
            tc.tile_pool(name="consts", bufs=1) as cpool,
            tc.tile_pool(name="work", bufs=wbufs) as wpool,
            tc.tile_pool(name="outs", bufs=6) as opool,
            tc.tile_pool(name="psum", bufs=8 if half_ev else 4,
                         space="PSUM") as ppool,
        ):
            # double-buffered inputs: rep r uses buffer set r%2, so a
            # rep's reloads WAR-wait on reads from TWO reps ago (a full
            # rep of slack) instead of serializing at the rep boundary
            # on the previous rep's last reader (mt is read by the very
            # last matmul)
            mts = [cpool.tile([P, KT, N], fp8, tag=f"mt{d}", name=f"mt{d}")
                   for d in range(2)]
            if swi:
                utss = [[
                    cpool.tile([P, 2, NBC, 2 * P], fp8, tag=f"ut{d}_{c}",
                               name=f"ut{d}_{c}")
                    for c in range(u_chunks)] for d in range(2)]
            else:
                utss = [[
                    cpool.tile([P, KT, CW], fp8, tag=f"ut{d}_{c}",
                               name=f"ut{d}_{c}")
                    for c in range(u_chunks)] for d in range(2)]
            # input loads issue on the idle gpsimd engine's DMA queue:
            # HWDGE is FIFO per ring, so sharing the SP ring queues
            # loads behind 64 y-stores; the ACT ring stalls W1s behind
            # the load's WAR semaphore; gpsimd's stream is free to block
            ldq = nc.gpsimd if lq_act else nc.sync

            def load_set(d):
                mt_d, uts_d = mts[d], utss[d]
                ldq.dma_start(mt_d[:], mat[:])
                for c in range(u_chunks):
                    if swi:
                        ldq.dma_start(
                            uts_d[c][:],
                            uT[:, :, c * NBC:(c + 1) * NBC, :])
                    else:
                        ldq.dma_start(
                            uts_d[c][:], uT[:, :, c * CW:(c + 1) * CW])

            for r in range(reps):
                d = (r % 2) if uload_in_reps else 0
                mt = mts[d]
                uts = utss[d]
                if r == 0 or uload_in_reps:
                    load_set(d)
                for b in range(NB):
                    i = r * NB + b
                    if half_ev:
                        pss = [ppool.tile([P, 512], f32, tag="ps",
                                          name=f"ps{h}") for h in range(2)]
                    else:
                        ps = ppool.tile([P, N], f32, tag="ps", name="ps")
                    if swi:
                        ut = uts[b // NBC]
                        bl = b % NBC
                    else:
                        ut = uts[(b * P) // CW]
                        boff = (b * P) % CW
                    for ks in (0, 2):
                        g = ks // 2
                        for h in range(2):
                            lhsT = (ut[:, g, bl, :] if swi
                                    else ut[:, ks:ks + 2, boff:boff + P])
                            out_ap = (pss[h][:] if half_ev
                                      else ps[:, h * 512:(h + 1) * 512])
                            nc.tensor.matmul(
                                out_ap,
                                lhsT,
                                mt[:, ks:ks + 2, h * 512:(h + 1) * 512],
                                start=(ks == 0),
                                stop=(ks == 2),
                                perf_mode=MODE,
                                skip_group_check=True,
                            )
                    if variant == "mm":
                        continue
                    t16 = wpool.tile([P, N], i16, tag="t16")
                    a16 = wpool.tile([P, N], i16, tag="a16")
                    if half_ev:
                        # per-half W1: each [P,512] psum bank is freed by
                        # its own op; ~2/3 of halves on ACT, rest DVE,
                        # spread evenly
                        for h in range(2):
                            j = 2 * i + h
                            if (j * 86) % 128 < 86:
                                nc.scalar.activation(
                                    t16[:, h * 512:(h + 1) * 512],
                                    pss[h][:],
                                    mybir.ActivationFunctionType.Copy,
                                )
                            else:
                                nc.vector.tensor_copy(
                                    t16[:, h * 512:(h + 1) * 512],
                                    pss[h][:],
                                )
                    # interleave the ACT/DVE W1 assignment evenly: a
                    # clustered split lets the busier engine fall behind
                    # the PE rate and stall psum recycling
                    elif (i * w1_act) % NB < w1_act:
                        nc.scalar.activation(
                            t16[:], ps[:],
                            mybir.ActivationFunctionType.Copy,
                        )
                    else:
                        nc.vector.tensor_copy(t16[:], ps[:])
                    nc.vector.tensor_scalar(
                        a16[:], t16[:], 1, None,
                        mybir.AluOpType.bitwise_and,
                    )
                    if evict8:
                        o8 = opool.tile([P, N], i8, tag="o8")
                        nc.vector.tensor_copy(o8[:], a16[:])
                        nc.sync.dma_start(y[b * P:(b + 1) * P, :], o8[:])
                    else:
                        nc.sync.dma_start(y[b * P:(b + 1) * P, :], a16[:])
    nc.compile()
    return nc


W1_ACT = 44
U_CHUNKS = 4
WBUFS = 12
SWI = True
REV = True
EVICT8 = False
LQ_ACT = True
HALF_EV = False


def get_nc(reps=1, uload_in_reps=False, variant="full"):
    key = (reps, W1_ACT, U_CHUNKS, WBUFS, uload_in_reps, SWI, REV, variant,
           EVICT8, LQ_ACT, HALF_EV)
    if key not in _nc_cache:
        _nc_cache[key] = _build_nc(reps, w1_act=W1_ACT, u_chunks=U_CHUNKS,
                                   wbufs=WBUFS, uload_in_reps=uload_in_reps,
                                   swi=SWI, rev=REV, variant=variant,
                                   evict8=EVICT8, lq_act=LQ_ACT,
                                   half_ev=HALF_EV)
    return _nc_cache[key]


def _to_k_major(a_km, free):
    return np.ascontiguousarray(
        a_km.reshape(KT, P, free).transpose(1, 0, 2)
    )


def make_swi(u3, rev=True):
    """[P, KT, SHARD] -> [P, 2, NB, 256] SwInterleave stationary layout."""
    u4 = u3.reshape(P, KT, NB, P)
    swi = np.empty((P, 2, NB, 2 * P), u3.dtype)
    for g in range(2):
        a = u4[:, 2 * g]
        bm = u4[:, 2 * g + 1]
        if rev:
            a = a[:, :, ::-1]
            bm = bm[:, :, ::-1]
        swi[:, g, :, 0::2] = a
        swi[:, g, :, 1::2] = bm
    return np.ascontiguousarray(swi)


def make_in_maps(u, M):
    u8 = np.asarray(u).astype(FP8_NP)
    m8 = np.asarray(M).astype(FP8_NP)
    mat3 = _to_k_major(m8, N)
    in_maps = []
    for i in range(N_CORES):
        uT_i = np.ascontiguousarray(u8[i * SHARD:(i + 1) * SHARD, :].T)
        u3 = _to_k_major(uT_i, SHARD)
        in_maps.append({"uT": make_swi(u3, REV) if SWI else u3,
                        "mat": mat3})
    return in_maps


def kernel(u, crc_gen, info_pos, ind_gather, perm_out):
    from concourse.bass_utils import run_bass_kernel_spmd

    M = build_M(crc_gen, info_pos, ind_gather, perm_out)
    in_maps = make_in_maps(u, M)
    nc = get_nc()
    res = run_bass_kernel_spmd(nc, in_maps, core_ids=list(range(N_CORES)))
    out = np.concatenate(
        [np.asarray(r["y"]).astype(np.float32) for r in res.results], axis=0
    )
    return out


# revision 17
# speedup vs baseline: 1.2774x; 1.0230x over previous
"""5G Polar encoder on 8 trn2 cores: one fused GF(2) matmul,
DoubleRowSwInterleave weights.

The whole reference computation is GF(2)-linear in u, so the host
composes one binary matrix M [512, 1024] from the tiny index tables and
the device computes y = (u @ M) mod 2, data-parallel over the batch
(8192 rows/core), as an fp8e4 matmul accumulating in f32 PSUM (exact:
sums <= 512), with mod-2 on the eviction path (ACT/DVE f32->i16, DVE
AND 1) and i16 {0,1} DMA'd out (host converts to f32).

Matmul perf mode is DoubleRowSwInterleave: the stationary operand is
stored flat [p, 256] with the two DR k-row-sets interleaved per column
and columns REVERSED (bass_interp.py:5260):
    F[p, 2*(127-m)]   = W0[p, m]   (k-row-set A = ks)
    F[p, 2*(127-m)+1] = W1[p, m]   (k-row-set B = ks+1)
so the PE reads weights contiguously.  Measured: the matmul stage drops
from ~75us (plain DoubleRow, LDWEIGHTS exposes ~80ns/mm since DR
disables fast weight load) to ~60-62us; full kernel ~76us vs ~79us.
Matmul order alternates PSUM banks (same-bank RMW back-to-back stalls)
and the stationary serves both psum halves.  Eviction split ACT 44 /
DVE 20 tiles balances both engines at ~45us, hidden under PE.  i8
output (extra DVE narrow) measured WORSE (80us) - engine time, not
HBM, is the secondary constraint.
"""

import numpy as np
import ml_dtypes

N_CORES = 8
BS = 65536
K = 512
N = 1024
SHARD = BS // N_CORES
P = 128
KT = K // P
NB = SHARD // P

FP8_NP = ml_dtypes.float8_e4m3

_nc_cache = {}


def build_M(crc_gen, info_pos, ind_gather, perm_out):
    crc_gen = np.asarray(crc_gen)
    info_pos = np.asarray(info_pos)
    ind_gather = np.asarray(ind_gather)
    perm_out = np.asarray(perm_out)
    k, _ = crc_gen.shape
    nb, n1 = ind_gather.shape
    kp = info_pos.shape[0]
    C = (crc_gen.astype(np.int64) & 1).astype(np.uint8)
    B = np.concatenate([np.eye(k, dtype=np.uint8), C], axis=1)
    col_src = np.full(n1, -1, np.int64)
    col_src[info_pos] = np.arange(kp)
    A = np.zeros((k, n1), np.uint8)
    valid = col_src >= 0
    A[:, valid] = B[:, col_src[valid]]
    for s in range(nb):
        A = A ^ A[:, ind_gather[s]]
    return A[:, perm_out]


def _build_nc(reps=1, w1_act=48, u_chunks=4, wbufs=6, uload_in_reps=False,
              swi=True, rev=True, variant="full", evict8=False,
              lq_act=True, half_ev=False, u_bounds=None):
    import concourse.tile as tile
    from concourse import bacc, mybir

    nc = bacc.Bacc("TRN2", target_bir_lowering=False, debug=False)
    fp8 = mybir.dt.float8e4
    f32 = mybir.dt.float32
    i16 = mybir.dt.int16
    i8 = mybir.dt.int8
    MODE = (mybir.MatmulPerfMode.DoubleRowSwInterleave if swi
            else mybir.MatmulPerfMode.DoubleRow)

    if swi:
        uT = nc.declare_dram_parameter("uT", [P, 2, NB, 2 * P], fp8,
                                       isOutput=False)
    else:
        uT = nc.declare_dram_parameter("uT", [P, KT, SHARD], fp8,
                                       isOutput=False)
    mat = nc.declare_dram_parameter("mat", [P, KT, N], fp8, isOutput=False)
    y = nc.declare_dram_parameter("y", [SHARD, N], i8 if evict8 else i16,
                                  isOutput=True)

    NBC = NB // u_chunks   # batch tiles per chunk (swi layout)
    CW = SHARD // u_chunks
    # tile-index boundaries of the u chunks (swi path); default uniform
    bnds = (list(u_bounds) if u_bounds is not None
            else [c * NBC for c in range(u_chunks)] + [NB])
    n_ch = len(bnds) - 1
    b2c = [next(c for c in range(n_ch) if bnds[c] <= b < bnds[c + 1])
           for b in range(NB)]
    with tile.TileContext(nc) as tc:
        with (
            tc.tile_pool(name="consts", bufs=1) as cpool,
            tc.tile_pool(name="work", bufs=wbufs) as wpool,
            tc.tile_pool(name="outs", bufs=6) as opool,
            tc.tile_pool(name="psum", bufs=8 if half_ev else 4,
                         space="PSUM") as ppool,
        ):
            # double-buffered inputs: rep r uses buffer set r%2, so a
            # rep's reloads WAR-wait on reads from TWO reps ago (a full
            # rep of slack) instead of serializing at the rep boundary
            # on the previous rep's last reader (mt is read by the very
            # last matmul)
            mts = [cpool.tile([P, KT, N], fp8, tag=f"mt{d}", name=f"mt{d}")
                   for d in range(2)]
            if swi:
                utss = [[
                    cpool.tile([P, 2, bnds[c + 1] - bnds[c], 2 * P], fp8,
                               tag=f"ut{d}_{c}", name=f"ut{d}_{c}")
                    for c in range(n_ch)] for d in range(2)]
            else:
                utss = [[
                    cpool.tile([P, KT, CW], fp8, tag=f"ut{d}_{c}",
                               name=f"ut{d}_{c}")
                    for c in range(u_chunks)] for d in range(2)]
            # input loads issue on the idle gpsimd engine's DMA queue:
            # HWDGE is FIFO per ring, so sharing the SP ring queues
            # loads behind 64 y-stores; the ACT ring stalls W1s behind
            # the load's WAR semaphore; gpsimd's stream is free to block
            ldq = nc.gpsimd if lq_act else nc.sync

            def load_set(d):
                mt_d, uts_d = mts[d], utss[d]
                ldq.dma_start(mt_d[:], mat[:])
                for c in range(n_ch if swi else u_chunks):
                    if swi:
                        ldq.dma_start(
                            uts_d[c][:],
                            uT[:, :, bnds[c]:bnds[c + 1], :])
                    else:
                        ldq.dma_start(
                            uts_d[c][:], uT[:, :, c * CW:(c + 1) * CW])

            for r in range(reps):
                d = (r % 2) if uload_in_reps else 0
                mt = mts[d]
                uts = utss[d]
                if r == 0 or uload_in_reps:
                    load_set(d)
                for b in range(NB):
                    i = r * NB + b
                    if half_ev:
                        pss = [ppool.tile([P, 512], f32, tag="ps",
                                          name=f"ps{h}") for h in range(2)]
                    else:
                        ps = ppool.tile([P, N], f32, tag="ps", name="ps")
                    if swi:
                        ut = uts[b2c[b]]
                        bl = b - bnds[b2c[b]]
                    else:
                        ut = uts[(b * P) // CW]
                        boff = (b * P) % CW
                    for ks in (0, 2):
                        g = ks // 2
                        for h in range(2):
                            lhsT = (ut[:, g, bl, :] if swi
                                    else ut[:, ks:ks + 2, boff:boff + P])
                            out_ap = (pss[h][:] if half_ev
                                      else ps[:, h * 512:(h + 1) * 512])
                            nc.tensor.matmul(
                                out_ap,
                                lhsT,
                                mt[:, ks:ks + 2, h * 512:(h + 1) * 512],
                                start=(ks == 0),
                                stop=(ks == 2),
                                perf_mode=MODE,
                                skip_group_check=True,
                            )
                    if variant == "mm":
                        continue
                    t16 = wpool.tile([P, N], i16, tag="t16")
                    a16 = wpool.tile([P, N], i16, tag="a16")
                    if half_ev:
                        # per-half W1: each [P,512] psum bank is freed by
                        # its own op; ~2/3 of halves on ACT, rest DVE,
                        # spread evenly
                        for h in range(2):
                            j = 2 * i + h
                            if (j * 86) % 128 < 86:
                                nc.scalar.activation(
                                    t16[:, h * 512:(h + 1) * 512],
                                    pss[h][:],
                                    mybir.ActivationFunctionType.Copy,
                                )
                            else:
                                nc.vector.tensor_copy(
                                    t16[:, h * 512:(h + 1) * 512],
                                    pss[h][:],
                                )
                    # interleave the ACT/DVE W1 assignment evenly: a
                    # clustered split lets the busier engine fall behind
                    # the PE rate and stall psum recycling
                    elif (i * w1_act) % NB < w1_act:
                        nc.scalar.activation(
                            t16[:], ps[:],
                            mybir.ActivationFunctionType.Copy,
                        )
                    else:
                        nc.vector.tensor_copy(t16[:], ps[:])
                    nc.vector.tensor_scalar(
                        a16[:], t16[:], 1, None,
                        mybir.AluOpType.bitwise_and,
                    )
                    if evict8:
                        o8 = opool.tile([P, N], i8, tag="o8")
                        nc.vector.tensor_copy(o8[:], a16[:])
                        nc.sync.dma_start(y[b * P:(b + 1) * P, :], o8[:])
                    else:
                        nc.sync.dma_start(y[b * P:(b + 1) * P, :], a16[:])
    nc.compile()
    return nc


W1_ACT = 44
U_CHUNKS = 4
WBUFS = 12
SWI = True
REV = True
EVICT8 = False
LQ_ACT = True
U_BOUNDS = None
HALF_EV = False


def get_nc(reps=1, uload_in_reps=False, variant="full"):
    key = (reps, W1_ACT, U_CHUNKS, WBUFS, uload_in_reps, SWI, REV, variant,
           EVICT8, LQ_ACT, HALF_EV,
           None if U_BOUNDS is None else tuple(U_BOUNDS))
    if key not in _nc_cache:
        _nc_cache[key] = _build_nc(reps, w1_act=W1_ACT, u_chunks=U_CHUNKS,
                                   wbufs=WBUFS, uload_in_reps=uload_in_reps,
                                   swi=SWI, rev=REV, variant=variant,
                                   evict8=EVICT8, lq_act=LQ_ACT,
                                   half_ev=HALF_EV, u_bounds=U_BOUNDS)
    return _nc_cache[key]


def _to_k_major(a_km, free):
    return np.ascontiguousarray(
        a_km.reshape(KT, P, free).transpose(1, 0, 2)
    )


def make_swi(u3, rev=True):
    """[P, KT, SHARD] -> [P, 2, NB, 256] SwInterleave stationary layout."""
    u4 = u3.reshape(P, KT, NB, P)
    swi = np.empty((P, 2, NB, 2 * P), u3.dtype)
    for g in range(2):
        a = u4[:, 2 * g]
        bm = u4[:, 2 * g + 1]
        if rev:
            a = a[:, :, ::-1]
            bm = bm[:, :, ::-1]
        swi[:, g, :, 0::2] = a
        swi[:, g, :, 1::2] = bm
    return np.ascontiguousarray(swi)


def make_in_maps(u, M):
    u8 = np.asarray(u).astype(FP8_NP)
    m8 = np.asarray(M).astype(FP8_NP)
    mat3 = _to_k_major(m8, N)
    in_maps = []
    for i in range(N_CORES):
        uT_i = np.ascontiguousarray(u8[i * SHARD:(i + 1) * SHARD, :].T)
        u3 = _to_k_major(uT_i, SHARD)
        in_maps.append({"uT": make_swi(u3, REV) if SWI else u3,
                        "mat": mat3})
    return in_maps


def kernel(u, crc_gen, info_pos, ind_gather, perm_out):
    from concourse.bass_utils import run_bass_kernel_spmd

    M = build_M(crc_gen, info_pos, ind_gather, perm_out)
    in_maps = make_in_maps(u, M)
    nc = get_nc()
    res = run_bass_kernel_spmd(nc, in_maps, core_ids=list(range(N_CORES)))
    out = np.concatenate(
        [np.asarray(r["y"]).astype(np.float32) for r in res.results], axis=0
    )
    return out
